# revision 35
# baseline (speedup 1.0000x reference)
"""MLA (DeepSeek-style multi-head latent attention) Bass kernel for 8 trn2 NeuronCores.

v2 design, bf16 compute:
- Stage 0 (sequence-sharded, 256 tokens/core): latents in [c, s] layout, kv
  c-tiles first so the small kv AllGather is issued early and overlaps the q
  c-tile compute; the bigger q AllGather overlaps the k/v projections.
- Stage 1 (tensor-parallel, 2 heads/core): k_nope/v from gathered kv latents,
  q from gathered q latents; rope on q/k_pe with host-folded signs.
- Attention per head with sb-paired N=512 streams (stationary weights reused
  across the pair), softmax without max-subtraction, denominator via
  ones-matmul, reciprocal broadcast on GpSimd.
- Output: AllToAll redistributes x from head-sharding to token-sharding, then
  each core computes its 256 output rows against the full wo (column streams),
  so there is no AllReduce and no 16 MB gather on the critical path.

All matmul operands bf16 (fp32 PSUM accumulation); norms/softmax stats fp32.
Host-side (free) preprocessing: weight transposes/permutations, norm and
softmax-scale folding, rope sign folding, bf16 casts.
"""

import math
import sys

import numpy as np

for _p in ("/opt/trn_rl_repo", "/root/.axon_site/_ro/trn_rl_repo"):
    if _p not in sys.path:
        sys.path.append(_p)

B, S, H = 1, 2048, 2048
NH = 16
Q_LORA, KV_LORA = 1536, 512
D_NOPE, D_ROPE, D_V = 128, 64, 128
D_QK = D_NOPE + D_ROPE
ROPE_FACTOR, MSCALE = 4.0, 1.0
SOFTMAX_SCALE = D_QK ** -0.5 * (0.1 * MSCALE * math.log(ROPE_FACTOR) + 1.0) ** 2
EPS = 1e-6

NCORES = 8
SSH = S // NCORES          # 256 tokens per core in stage 0

_CACHE = {}


def _build(has_mask: bool):
    import concourse.bacc as bacc
    import concourse.mybir as mybir
    import concourse.tile as tile

    bf = mybir.dt.bfloat16
    f32 = mybir.dt.float32
    f32r = mybir.dt.float32r
    AF = mybir.ActivationFunctionType
    OP = mybir.AluOpType

    nc = bacc.Bacc("TRN2", target_bir_lowering=False, debug=False,
                   num_devices=NCORES)

    hidT = nc.dram_tensor("hidT", [16, 128, SSH], bf, kind="ExternalInput")
    a_t = nc.dram_tensor("a_t", [16, 128, 2112], bf, kind="ExternalInput")
    cosT_sh = nc.dram_tensor("cosT_sh", [64, SSH], f32, kind="ExternalInput")
    sinTs_sh = nc.dram_tensor("sinTs_sh", [64, SSH], f32, kind="ExternalInput")
    cosT2 = nc.dram_tensor("cosT2", [128, S], f32, kind="ExternalInput")
    sinT2s = nc.dram_tensor("sinT2s", [128, S], f32, kind="ExternalInput")
    wqbT = nc.dram_tensor("wqbT", [12, 128, 384], bf, kind="ExternalInput")
    wkvbT = nc.dram_tensor("wkvbT", [4, 128, 512], bf, kind="ExternalInput")
    woT = nc.dram_tensor("woT", [16, 128, S], bf, kind="ExternalInput")
    ones_bf = nc.dram_tensor("ones_bf", [128, 1], bf, kind="ExternalInput")
    ones_fr = nc.dram_tensor("ones_fr", [128, 1], f32r, kind="ExternalInput")
    onesr_fr = nc.dram_tensor("onesr_fr", [1, 128], f32r, kind="ExternalInput")
    if has_mask:
        maskT = nc.dram_tensor("maskT", [S, S], f32, kind="ExternalInput")
    out = nc.dram_tensor("out", [SSH, S], f32, kind="ExternalOutput")

    # partition-major bounce layouts: stores and post-gather loads are both
    # contiguous multi-KB lines per partition (no transposing descriptors)
    bounce_kv = nc.dram_tensor("bounce_kv", [128, 5, SSH], bf)
    gath_kv = nc.dram_tensor("gath_kv", [NCORES, 128, 5, SSH], bf)
    bounce_q = nc.dram_tensor("bounce_q", [128, 12, SSH], bf)
    gath_q = nc.dram_tensor("gath_q", [NCORES, 128, 12, SSH], bf)
    bounce_x0 = nc.dram_tensor("bounce_x0", [NCORES, 128, SSH], bf)
    gath_x0 = nc.dram_tensor("gath_x0", [NCORES, 128, SSH], bf)
    bounce_x1 = nc.dram_tensor("bounce_x1", [NCORES, 128, SSH], bf)
    gath_x1 = nc.dram_tensor("gath_x1", [NCORES, 128, SSH], bf)

    RG = [list(range(NCORES))]

    def mm(ps, lhsT, rhs, start, stop):
        nc.tensor.matmul(ps, lhsT, rhs, start=start, stop=stop)

    from contextlib import ExitStack
    with tile.TileContext(nc) as tc, ExitStack() as _st:
        constp = _st.enter_context(tc.tile_pool(name="const", bufs=1))
        ones_c_bf = constp.tile([128, 1], bf)
        nc.sync.dma_start(ones_c_bf[:], ones_bf.ap())
        ones_c_fr = constp.tile([128, 1], f32r)
        nc.sync.dma_start(ones_c_fr[:], ones_fr.ap())
        ones_r_fr = constp.tile([1, 128], f32r)
        nc.sync.dma_start(ones_r_fr[:], onesr_fr.ap())
        eps_sb = constp.tile([1, 1], f32)
        nc.any.memset(eps_sb[:], EPS)

        # ---------------- stage 0: latents for own 256 tokens, [c, s] layout
        with tc.tile_pool(name="s0", bufs=1) as s0p, \
             tc.tile_pool(name="s0ps", bufs=3, space="PSUM") as s0ps, \
             tc.tile_pool(name="s0ss", bufs=1, space="PSUM") as s0ssp, \
             tc.tile_pool(name="s0pb", bufs=2, space="PSUM") as s0pb, \
             tc.tile_pool(name="s0sq", bufs=3) as s0sqp:
            hid_sb = s0p.tile([128, 16, SSH], bf)
            nc.sync.dma_start(hid_sb[:], hidT.ap().rearrange("o p s -> p o s"))
            a_kv_sb = s0p.tile([128, 16, 576], bf)
            nc.sync.dma_start(
                a_kv_sb[:], a_t.ap()[:, :, 1536:2112].rearrange("o p c -> p o c"))
            a_q_sb = s0p.tile([128, 16, 1536], bf)
            for c0 in (0, 512, 1024):
                nc.sync.dma_start(
                    a_q_sb[:, :, c0:c0 + 512],
                    a_t.ap()[:, :, c0:c0 + 512].rearrange("o p c -> p o c"))

            # kv c-tiles (4 full + kpe 64); sumsq matmul deferred one tile so
            # the PE never waits on the ACT Square
            raw_kv = s0p.tile([128, 5, SSH], f32)
            ss_kv = s0ssp.tile([1, SSH], f32)
            pend_sq = None
            nsq = 0
            for i in range(5):
                w = 128 if i < 4 else 64
                ps = s0ps.tile([128, SSH], f32, tag="s0ps")
                for hb in range(16):
                    mm(ps[:w], a_kv_sb[:, hb, i * 128:i * 128 + w],
                       hid_sb[:, hb, :], hb == 0, hb == 15)
                if pend_sq is not None:
                    mm(ss_kv, ones_c_fr, pend_sq, nsq == 0, nsq == 3)
                    nsq += 1
                    pend_sq = None
                nc.vector.tensor_copy(raw_kv[:w, i, :], ps[:w])
                if i < 4:
                    sq = s0sqp.tile([128, SSH], f32r, tag="s0sq")
                    nc.scalar.activation(sq[:], ps[:], AF.Square)
                    pend_sq = sq

            sq_kv = s0p.tile([1, SSH], f32)
            nc.scalar.activation(sq_kv[:], ss_kv[:], AF.Sqrt,
                                 bias=eps_sb[:], scale=1.0 / KV_LORA)
            rc_kv = s0p.tile([1, SSH], f32r)
            with nc.allow_low_precision(reason="f32r rms scale is fine"):
                nc.vector.reciprocal(rc_kv[:], sq_kv[:])
            psb_kv = s0pb.tile([128, SSH], f32, tag="s0pb")
            mm(psb_kv, ones_r_fr, rc_kv, True, True)
            bc_kv = s0p.tile([128, SSH], f32)
            nc.scalar.copy(bc_kv[:], psb_kv[:])

            lat_kv = s0p.tile([128, 5, SSH], bf)
            for i in range(4):
                nc.vector.tensor_tensor(lat_kv[:, i, :], raw_kv[:, i, :],
                                        bc_kv[:], OP.mult)
            # k_pe rope (not normalized); rows [0:64) of tile 4
            cs_sb = s0p.tile([64, SSH], f32)
            nc.sync.dma_start(cs_sb[:], cosT_sh.ap())
            sn_sb = s0p.tile([64, SSH], f32)
            nc.sync.dma_start(sn_sb[:], sinTs_sh.ap())
            rsw = s0p.tile([64, SSH], f32)
            nc.sync.dma_start(rsw[0:32], raw_kv[32:64, 4, :])
            nc.sync.dma_start(rsw[32:64], raw_kv[0:32, 4, :])
            t1 = s0p.tile([64, SSH], f32)
            nc.vector.tensor_tensor(t1[:], raw_kv[0:64, 4, :], cs_sb[:],
                                    OP.mult)
            nc.vector.tensor_tensor(rsw[:], rsw[:], sn_sb[:], OP.mult)
            nc.vector.tensor_tensor(lat_kv[0:64, 4, :], t1[:], rsw[:], OP.add)
            nc.any.memset(lat_kv[64:128, 4, :], 0.0)
            nc.sync.dma_start(bounce_kv.ap(), lat_kv[:])

            nc.gpsimd.collective_compute(
                "AllGather", OP.bypass, replica_groups=RG,
                ins=[bounce_kv.ap().opt()], outs=[gath_kv.ap().opt()])

            # q c-tiles (sumsq deferred one tile, as above)
            raw_q = s0p.tile([128, 12, SSH], f32)
            ss_hq = s0ssp.tile([1, SSH], f32)
            pend_sq = None
            nsq = 0
            for ct in range(12):
                ps = s0ps.tile([128, SSH], f32, tag="s0ps")
                for hb in range(16):
                    mm(ps, a_q_sb[:, hb, ct * 128:(ct + 1) * 128],
                       hid_sb[:, hb, :], hb == 0, hb == 15)
                if pend_sq is not None:
                    mm(ss_hq, ones_c_fr, pend_sq, nsq == 0, nsq == 11)
                    nsq += 1
                nc.vector.tensor_copy(raw_q[:, ct, :], ps[:])
                sq = s0sqp.tile([128, SSH], f32r, tag="s0sq")
                nc.scalar.activation(sq[:], ps[:], AF.Square)
                pend_sq = sq
            mm(ss_hq, ones_c_fr, pend_sq, nsq == 0, nsq == 11)

            sq_hq = s0p.tile([1, SSH], f32)
            nc.scalar.activation(sq_hq[:], ss_hq[:], AF.Sqrt,
                                 bias=eps_sb[:], scale=1.0 / Q_LORA)
            rc_hq = s0p.tile([1, SSH], f32r)
            with nc.allow_low_precision(reason="f32r rms scale is fine"):
                nc.vector.reciprocal(rc_hq[:], sq_hq[:])
            psb_hq = s0pb.tile([128, SSH], f32, tag="s0pb")
            mm(psb_hq, ones_r_fr, rc_hq, True, True)
            bc_hq = s0p.tile([128, SSH], f32)
            nc.scalar.copy(bc_hq[:], psb_hq[:])

            lat_q = s0p.tile([128, 12, SSH], bf)
            for ct in range(12):
                nc.vector.tensor_tensor(lat_q[:, ct, :], raw_q[:, ct, :],
                                        bc_hq[:], OP.mult)
            nc.sync.dma_start(bounce_q.ap(), lat_q[:])

            nc.gpsimd.collective_compute(
                "AllGather", OP.bypass, replica_groups=RG,
                ins=[bounce_q.ap().opt()], outs=[gath_q.ap().opt()])

        # ---------------- stage 1: per-head projections + attention
        with tc.tile_pool(name="s1w", bufs=1) as s1w, \
             tc.tile_pool(name="att", bufs=1) as attp:
            wqb_sb = s1w.tile([128, 12, 384], bf)
            nc.sync.dma_start(wqb_sb[:], wqbT.ap().rearrange("o p d -> p o d"))
            kpe_sb = attp.tile([64, S], bf)
            for r in range(NCORES):
                nc.sync.dma_start(kpe_sb[:, r * SSH:(r + 1) * SSH],
                                  gath_kv.ap()[r, 0:64, 4, :])

            kn_sb = s1w.tile([128, 2, S], bf)
            vt_sb = s1w.tile([128, 16, 256], bf)
            qn0 = attp.tile([128, S], bf)
            qn1 = attp.tile([128, S], bf)
            qt1 = attp.tile([128, S], f32)
            qrb = attp.tile([128, S], bf)
            qr1 = attp.tile([64, S], bf)
            cos2_sb = attp.tile([128, S], f32)
            nc.scalar.dma_start(cos2_sb[:], cosT2.ap())
            sin2_sb = attp.tile([128, S], f32)
            nc.scalar.dma_start(sin2_sb[:], sinT2s.ap())

            # k/v projections in a scoped pool so their inputs free before
            # the q path needs the SBUF
            with tc.tile_pool(name="s1kv", bufs=1) as kvp, \
                 tc.tile_pool(name="p1ps", bufs=3, space="PSUM") as p1ps, \
                 tc.tile_pool(name="p1psv", bufs=2, space="PSUM") as p1psv:
                wkvb_sb = kvp.tile([128, 4, 512], bf)
                nc.sync.dma_start(wkvb_sb[:],
                                  wkvbT.ap().rearrange("o p d -> p o d"))
                # gather load in [c_part, r, cc, s] layout (contiguous lines)
                kv_sb = kvp.tile([128, 8, 4, SSH], bf)
                for r in range(NCORES):
                    nc.sync.dma_start(kv_sb[:, r], gath_kv.ap()[r, :, 0:4, :])
                # k_nope for the 2 own heads
                for kh in range(2):
                    for tch in range(4):
                        ps = p1ps.tile([128, 512], f32, tag="p1ps")
                        for cc in range(4):
                            mm(ps, wkvb_sb[:, cc, kh * 128:(kh + 1) * 128],
                               kv_sb[:, 2 * tch:2 * tch + 2, cc, :],
                               cc == 0, cc == 3)
                        nc.scalar.copy(kn_sb[:, kh, tch * 512:(tch + 1) * 512],
                                       ps[:])
                # v^T tiles [t, 2*128]
                for tb in range(16):
                    ps = p1psv.tile([128, 256], f32, tag="p1psv")
                    for cc in range(4):
                        mm(ps, kv_sb[:, tb // 2, cc,
                                     (tb % 2) * 128:(tb % 2) * 128 + 128],
                           wkvb_sb[:, cc, 256:512], cc == 0, cc == 3)
                    nc.vector.tensor_copy(vt_sb[:, tb, :], ps[:])

            # q projections (needs the q AllGather); rope fused per chunk
            with tc.tile_pool(name="p1q", bufs=3, space="PSUM") as p1ps:
                with tc.tile_pool(name="hq", bufs=1) as hqp, \
                     tc.tile_pool(name="ropet", bufs=2) as ropep:
                    hq_sb = hqp.tile([128, 8, 12, SSH], bf)
                    for r in range(NCORES):
                        nc.scalar.dma_start(hq_sb[:, r], gath_q.ap()[r])
                    qdst = (qn0, qt1, qn1)
                    for m in range(3):
                        for tch in range(4):
                            c0, c1 = tch * 512, (tch + 1) * 512
                            ps = p1ps.tile([128, 512], f32, tag="p1ps")
                            for cc in range(12):
                                mm(ps, wqb_sb[:, cc, m * 128:(m + 1) * 128],
                                   hq_sb[:, 2 * tch:2 * tch + 2, cc, :],
                                   cc == 0, cc == 11)
                            if m == 1:
                                nc.vector.tensor_copy(qt1[:, c0:c1], ps[:])
                                # rope on this 512-token chunk
                                tmp = ropep.tile([128, 512], f32, tag="tmp")
                                for b in (0, 64):
                                    nc.sync.dma_start(
                                        tmp[b:b + 32, :],
                                        qt1[b + 32:b + 64, c0:c1])
                                    nc.sync.dma_start(
                                        tmp[b + 32:b + 64, :],
                                        qt1[b:b + 32, c0:c1])
                                nc.vector.tensor_tensor(
                                    qt1[:, c0:c1], qt1[:, c0:c1],
                                    cos2_sb[:, c0:c1], OP.mult)
                                nc.vector.tensor_tensor(
                                    tmp[:, :], tmp[:, :],
                                    sin2_sb[:, c0:c1], OP.mult)
                                nc.vector.tensor_tensor(
                                    qrb[:, c0:c1], qt1[:, c0:c1],
                                    tmp[:, :], OP.add)
                                nc.sync.dma_start(qr1[:, c0:c1],
                                                  qrb[64:128, c0:c1])
                            else:
                                nc.scalar.copy(qdst[m][:, c0:c1], ps[:])

            # full wo, loaded during attention (bulk ring)
            wo_sb = s1w.tile([128, 16, S], bf)
            nc.scalar.dma_start(wo_sb[:], woT.ap().rearrange("o p s -> p o s"))

            # attention, 2 heads, sb-paired 512-token column chunks
            with tc.tile_pool(name="apss", bufs=4, space="PSUM") as apss, \
                 tc.tile_pool(name="apsx", bufs=2, space="PSUM") as apsx, \
                 tc.tile_pool(name="apsd", bufs=2, space="PSUM") as apsd, \
                 tc.tile_pool(name="aex", bufs=6) as aexp, \
                 tc.tile_pool(name="asm", bufs=4) as asmp, \
                 tc.tile_pool(name="amk", bufs=4) as amkp, \
                 tc.tile_pool(name="xh", bufs=1) as xhp:
                for h in range(2):
                    qn_h = qn0 if h == 0 else qn1
                    qr_h = qrb if h == 0 else qr1
                    xh = xhp.tile([128, S], bf, name=f"xh{h}")
                    for sbp in range(2):
                        sA = 2 * sbp
                        sB = sA + 1
                        psx = [apsx.tile([128, 512], f32, tag="apsx",
                                         name=f"psx{j}")
                               for j in range(2)]
                        psd = [apsd.tile([1, 512], f32, tag="apsd",
                                         name=f"psd{j}")
                               for j in range(2)]
                        # software pipeline: scores for tb stream while the
                        # av/denominator matmuls consume ex[tb-1], so the PE
                        # never waits for the ACT exp
                        pend_ex = None
                        for tb in range(16):
                            pss = [apss.tile([128, 512], f32, tag="apss",
                                             name=f"pss{j}")
                                   for j in range(2)]
                            for j, sb in enumerate((sA, sB)):
                                mm(pss[j], kn_sb[:, h, tb * 128:(tb + 1) * 128],
                                   qn_h[:, sb * 512:(sb + 1) * 512],
                                   True, False)
                            for j, sb in enumerate((sA, sB)):
                                mm(pss[j], kpe_sb[:, tb * 128:(tb + 1) * 128],
                                   qr_h[0:64, sb * 512:(sb + 1) * 512],
                                   False, True)
                            if pend_ex is not None:
                                for j in range(2):
                                    mm(psx[j],
                                       vt_sb[:, tb - 1, h * 128:(h + 1) * 128],
                                       pend_ex[j], tb == 1, False)
                                for j in range(2):
                                    mm(psd[j], ones_c_bf, pend_ex[j],
                                       tb == 1, False)
                            ex = []
                            for j, sb in enumerate((sA, sB)):
                                if has_mask:
                                    mk = amkp.tile([128, 512], f32, tag="amk")
                                    nc.sync.dma_start(
                                        mk[:],
                                        maskT.ap()[tb * 128:(tb + 1) * 128,
                                                   sb * 512:(sb + 1) * 512])
                                    nc.vector.tensor_tensor(
                                        pss[j][:], pss[j][:], mk[:], OP.add)
                                e = aexp.tile([128, 512], bf, tag="aex",
                                              name=f"ex{j}")
                                nc.scalar.activation(e[:], pss[j][:], AF.Exp)
                                ex.append(e)
                            pend_ex = ex
                        for j in range(2):
                            mm(psx[j], vt_sb[:, 15, h * 128:(h + 1) * 128],
                               pend_ex[j], False, True)
                        for j in range(2):
                            mm(psd[j], ones_c_bf, pend_ex[j], False, True)
                        for j, sb in enumerate((sA, sB)):
                            rd = asmp.tile([1, 512], f32r, tag="rd")
                            with nc.allow_low_precision(
                                    reason="softmax denom reciprocal"):
                                nc.vector.reciprocal(rd[:], psd[j][:])
                            psb = apss.tile([128, 512], f32, tag="apss",
                                            name="psbc")
                            mm(psb, ones_r_fr, rd, True, True)
                            bcd = asmp.tile([128, 512], f32, tag="bcd")
                            nc.scalar.copy(bcd[:], psb[:])
                            nc.vector.tensor_tensor(
                                xh[:, sb * 512:(sb + 1) * 512], psx[j][:],
                                bcd[:], OP.mult)
                    bx = bounce_x0 if h == 0 else bounce_x1
                    for j in range(NCORES):
                        nc.sync.dma_start(bx.ap()[j],
                                          xh[:, j * SSH:(j + 1) * SSH])
                    # per-head AllToAll: head 0's redistribution overlaps
                    # head 1's attention
                    nc.gpsimd.collective_compute(
                        "AllToAll", OP.bypass, replica_groups=RG,
                        ins=[bx.ap().opt()],
                        outs=[(gath_x0 if h == 0 else gath_x1).ap().opt()])

            # ---------------- output projection (token-sharded rows),
            # two passes over held PSUM groups: head-0 channels can start
            # while head-1's AllToAll is still in flight
            with tc.tile_pool(name="wops", bufs=1, space="PSUM") as wops, \
                 tc.tile_pool(name="woot", bufs=3) as wootp:
                xg0_sb = attp.tile([128, NCORES, SSH], bf)
                nc.sync.dma_start(
                    xg0_sb[:], gath_x0.ap().rearrange("r p s -> p r s"))
                pso = [wops.tile([128, 512], f32, name=f"pso{k}")
                       for k in range(8)]
                for st2 in range(2):
                    for hc in range(4):
                        for r in range(NCORES):
                            mm(pso[st2 * 4 + hc],
                               xg0_sb[:, r, st2 * 128:(st2 + 1) * 128],
                               wo_sb[:, 2 * r, hc * 512:(hc + 1) * 512],
                               r == 0, False)
                xg1_sb = attp.tile([128, NCORES, SSH], bf)
                nc.sync.dma_start(
                    xg1_sb[:], gath_x1.ap().rearrange("r p s -> p r s"))
                for st2 in range(2):
                    for hc in range(4):
                        for r in range(NCORES):
                            mm(pso[st2 * 4 + hc],
                               xg1_sb[:, r, st2 * 128:(st2 + 1) * 128],
                               wo_sb[:, 2 * r + 1, hc * 512:(hc + 1) * 512],
                               False, r == NCORES - 1)
                        ot = wootp.tile([128, 512], f32, tag="ot")
                        nc.scalar.copy(ot[:], pso[st2 * 4 + hc][:])
                        nc.sync.dma_start(
                            out.ap()[st2 * 128:(st2 + 1) * 128,
                                     hc * 512:(hc + 1) * 512], ot[:])

    nc.compile()
    return nc


def _prep_inputs(hidden_states, cos, sin, attn_mask, wq_a, q_norm_w, wq_b,
                 wkv_a, kv_norm_w, wkv_b, wo, has_mask):
    import ml_dtypes
    bf16 = ml_dtypes.bfloat16
    c = np.ascontiguousarray

    hid = np.asarray(hidden_states, np.float32)[0]          # [S, H]
    hidT = hid.T.astype(bf16)                               # [H, S]
    A_T = np.vstack([np.asarray(wq_a, np.float32),
                     np.asarray(wkv_a, np.float32)]).T      # [H, 2112]
    a_t = c(A_T.astype(bf16).reshape(16, 128, 2112))

    cosT = np.asarray(cos, np.float32).T                    # [64, S]
    sinT = np.asarray(sin, np.float32).T
    sinTs = sinT.copy()
    sinTs[0:32] *= -1.0
    cosT2 = c(np.concatenate([cosT, cosT], 0))              # [128, S]
    sinT2s = c(np.concatenate([sinTs, sinTs], 0))

    wqb = np.asarray(wq_b, np.float32) * np.asarray(q_norm_w, np.float32)[None]
    wqb = wqb * SOFTMAX_SCALE
    wkvb = (np.asarray(wkv_b, np.float32)
            * np.asarray(kv_norm_w, np.float32)[None])
    woT_full = c(np.asarray(wo, np.float32).T.astype(bf16)
                 .reshape(16, 128, S))                      # [NH*DV, H]

    qperm = np.r_[0:128, 128:192, 320:384, 192:320]
    kvperm = np.r_[0:128, 256:384, 128:256, 384:512]

    in_maps = []
    for r in range(NCORES):
        m = {
            "hidT": c(hidT[:, r * SSH:(r + 1) * SSH].reshape(16, 128, SSH)),
            "a_t": a_t,
            "cosT_sh": c(cosT[:, r * SSH:(r + 1) * SSH]),
            "sinTs_sh": c(sinTs[:, r * SSH:(r + 1) * SSH]),
            "cosT2": cosT2,
            "sinT2s": sinT2s,
            "wqbT": c(wqb[r * 384:(r + 1) * 384].T[:, qperm]
                      .astype(bf16).reshape(12, 128, 384)),
            "wkvbT": c(wkvb[r * 512:(r + 1) * 512].T[:, kvperm]
                       .astype(bf16).reshape(4, 128, 512)),
            "woT": woT_full,
            "ones_bf": np.ones((128, 1), bf16),
            "ones_fr": np.ones((128, 1), np.float32),
            "onesr_fr": np.ones((1, 128), np.float32),
        }
        if has_mask:
            m["maskT"] = c(np.asarray(attn_mask, np.float32).T)
        in_maps.append(m)
    return in_maps


def assemble(res):
    full = np.concatenate([np.asarray(res[r]["out"], np.float32)
                           for r in range(NCORES)], axis=0)
    return full.reshape(B, S, H)


def kernel(**inputs):
    from concourse.bass_utils import run_bass_kernel_spmd

    has_mask = bool(np.any(np.asarray(inputs["attn_mask"])))
    if has_mask not in _CACHE:
        _CACHE[has_mask] = _build(has_mask)
    nc = _CACHE[has_mask]

    in_maps = _prep_inputs(has_mask=has_mask, **inputs)
    res = run_bass_kernel_spmd(nc, in_maps, list(range(NCORES))).results
    return assemble(res)


# revision 39
# speedup vs baseline: 1.1287x; 1.1287x over previous
"""MLA (DeepSeek-style multi-head latent attention) Bass kernel for 8 trn2 NeuronCores.

v2 design, bf16 compute:
- Stage 0 (sequence-sharded, 256 tokens/core): latents in [c, s] layout, kv
  c-tiles first so the small kv AllGather is issued early and overlaps the q
  c-tile compute; the bigger q AllGather overlaps the k/v projections.
- Stage 1 (tensor-parallel, 2 heads/core): k_nope/v from gathered kv latents,
  q from gathered q latents; rope on q/k_pe with host-folded signs.
- Attention per head with sb-paired N=512 streams (stationary weights reused
  across the pair), softmax without max-subtraction, denominator via
  ones-matmul, reciprocal broadcast on GpSimd.
- Output: AllToAll redistributes x from head-sharding to token-sharding, then
  each core computes its 256 output rows against the full wo (column streams),
  so there is no AllReduce and no 16 MB gather on the critical path.

All matmul operands bf16 (fp32 PSUM accumulation); norms/softmax stats fp32.
Host-side (free) preprocessing: weight transposes/permutations, norm and
softmax-scale folding, rope sign folding, bf16 casts.
"""

import math
import sys

import numpy as np

for _p in ("/opt/trn_rl_repo", "/root/.axon_site/_ro/trn_rl_repo"):
    if _p not in sys.path:
        sys.path.append(_p)

B, S, H = 1, 2048, 2048
NH = 16
Q_LORA, KV_LORA = 1536, 512
D_NOPE, D_ROPE, D_V = 128, 64, 128
D_QK = D_NOPE + D_ROPE
ROPE_FACTOR, MSCALE = 4.0, 1.0
SOFTMAX_SCALE = D_QK ** -0.5 * (0.1 * MSCALE * math.log(ROPE_FACTOR) + 1.0) ** 2
EPS = 1e-6

NCORES = 8
SSH = S // NCORES          # 256 tokens per core in stage 0

_CACHE = {}


def _build(has_mask: bool):
    import concourse.bacc as bacc
    import concourse.mybir as mybir
    import concourse.tile as tile

    bf = mybir.dt.bfloat16
    f32 = mybir.dt.float32
    f32r = mybir.dt.float32r
    AF = mybir.ActivationFunctionType
    OP = mybir.AluOpType

    nc = bacc.Bacc("TRN2", target_bir_lowering=False, debug=False,
                   num_devices=NCORES)

    hidT = nc.dram_tensor("hidT", [16, 128, SSH], bf, kind="ExternalInput")
    a_t = nc.dram_tensor("a_t", [16, 128, 2112], bf, kind="ExternalInput")
    cosT_sh = nc.dram_tensor("cosT_sh", [64, SSH], f32, kind="ExternalInput")
    sinTs_sh = nc.dram_tensor("sinTs_sh", [64, SSH], f32, kind="ExternalInput")
    cosT2 = nc.dram_tensor("cosT2", [128, S], f32, kind="ExternalInput")
    sinT2s = nc.dram_tensor("sinT2s", [128, S], f32, kind="ExternalInput")
    wqbT = nc.dram_tensor("wqbT", [12, 128, 384], bf, kind="ExternalInput")
    wkvbT = nc.dram_tensor("wkvbT", [4, 128, 512], bf, kind="ExternalInput")
    woT = nc.dram_tensor("woT", [16, 128, S], bf, kind="ExternalInput")
    ones_bf = nc.dram_tensor("ones_bf", [128, 1], bf, kind="ExternalInput")
    ones_fr = nc.dram_tensor("ones_fr", [128, 1], f32r, kind="ExternalInput")
    onesr_fr = nc.dram_tensor("onesr_fr", [1, 128], f32r, kind="ExternalInput")
    if has_mask:
        maskT = nc.dram_tensor("maskT", [S, S], f32, kind="ExternalInput")
    out = nc.dram_tensor("out", [SSH, S], f32, kind="ExternalOutput")

    # partition-major bounce layouts: stores and post-gather loads are both
    # contiguous multi-KB lines per partition (no transposing descriptors)
    bounce_kv = nc.dram_tensor("bounce_kv", [128, 5, SSH], bf)
    gath_kv = nc.dram_tensor("gath_kv", [NCORES, 128, 5, SSH], bf,
                             addr_space="Shared")
    bounce_q = nc.dram_tensor("bounce_q", [128, 12, SSH], bf)
    gath_q = nc.dram_tensor("gath_q", [NCORES, 128, 12, SSH], bf,
                            addr_space="Shared")
    bounce_x0 = nc.dram_tensor("bounce_x0", [NCORES, 128, SSH], bf)
    gath_x0 = nc.dram_tensor("gath_x0", [NCORES, 128, SSH], bf)
    bounce_x1 = nc.dram_tensor("bounce_x1", [NCORES, 128, SSH], bf)
    gath_x1 = nc.dram_tensor("gath_x1", [NCORES, 128, SSH], bf)

    RG = [list(range(NCORES))]

    def mm(ps, lhsT, rhs, start, stop):
        nc.tensor.matmul(ps, lhsT, rhs, start=start, stop=stop)

    from contextlib import ExitStack
    with tile.TileContext(nc) as tc, ExitStack() as _st:
        constp = _st.enter_context(tc.tile_pool(name="const", bufs=1))
        ones_c_bf = constp.tile([128, 1], bf)
        nc.sync.dma_start(ones_c_bf[:], ones_bf.ap())
        ones_c_fr = constp.tile([128, 1], f32r)
        nc.sync.dma_start(ones_c_fr[:], ones_fr.ap())
        ones_r_fr = constp.tile([1, 128], f32r)
        nc.sync.dma_start(ones_r_fr[:], onesr_fr.ap())
        eps_sb = constp.tile([1, 1], f32)
        nc.any.memset(eps_sb[:], EPS)

        # ---------------- stage 0: latents for own 256 tokens, [c, s] layout
        with tc.tile_pool(name="s0", bufs=1) as s0p, \
             tc.tile_pool(name="s0ps", bufs=3, space="PSUM") as s0ps, \
             tc.tile_pool(name="s0ss", bufs=1, space="PSUM") as s0ssp, \
             tc.tile_pool(name="s0pb", bufs=2, space="PSUM") as s0pb, \
             tc.tile_pool(name="s0sq", bufs=3) as s0sqp:
            hid_sb = s0p.tile([128, 16, SSH], bf)
            nc.sync.dma_start(hid_sb[:], hidT.ap().rearrange("o p s -> p o s"))
            a_kv_sb = s0p.tile([128, 16, 576], bf)
            nc.sync.dma_start(
                a_kv_sb[:], a_t.ap()[:, :, 1536:2112].rearrange("o p c -> p o c"))
            a_q_sb = s0p.tile([128, 16, 1536], bf)
            for c0 in (0, 512, 1024):
                nc.sync.dma_start(
                    a_q_sb[:, :, c0:c0 + 512],
                    a_t.ap()[:, :, c0:c0 + 512].rearrange("o p c -> p o c"))

            # kv c-tiles (4 full + kpe 64); sumsq matmul deferred one tile so
            # the PE never waits on the ACT Square
            raw_kv = s0p.tile([128, 5, SSH], f32)
            ss_kv = s0ssp.tile([1, SSH], f32)
            pend_sq = None
            nsq = 0
            for i in range(5):
                w = 128 if i < 4 else 64
                ps = s0ps.tile([128, SSH], f32, tag="s0ps")
                for hb in range(16):
                    mm(ps[:w], a_kv_sb[:, hb, i * 128:i * 128 + w],
                       hid_sb[:, hb, :], hb == 0, hb == 15)
                if pend_sq is not None:
                    mm(ss_kv, ones_c_fr, pend_sq, nsq == 0, nsq == 3)
                    nsq += 1
                    pend_sq = None
                nc.vector.tensor_copy(raw_kv[:w, i, :], ps[:w])
                if i < 4:
                    sq = s0sqp.tile([128, SSH], f32r, tag="s0sq")
                    nc.scalar.activation(sq[:], ps[:], AF.Square)
                    pend_sq = sq

            sq_kv = s0p.tile([1, SSH], f32)
            nc.scalar.activation(sq_kv[:], ss_kv[:], AF.Sqrt,
                                 bias=eps_sb[:], scale=1.0 / KV_LORA)
            rc_kv = s0p.tile([1, SSH], f32r)
            with nc.allow_low_precision(reason="f32r rms scale is fine"):
                nc.vector.reciprocal(rc_kv[:], sq_kv[:])
            psb_kv = s0pb.tile([128, SSH], f32, tag="s0pb")
            mm(psb_kv, ones_r_fr, rc_kv, True, True)
            bc_kv = s0p.tile([128, SSH], f32)
            nc.scalar.copy(bc_kv[:], psb_kv[:])

            lat_kv = s0p.tile([128, 5, SSH], bf)
            for i in range(4):
                nc.vector.tensor_tensor(lat_kv[:, i, :], raw_kv[:, i, :],
                                        bc_kv[:], OP.mult)
            # k_pe rope (not normalized); rows [0:64) of tile 4
            cs_sb = s0p.tile([64, SSH], f32)
            nc.sync.dma_start(cs_sb[:], cosT_sh.ap())
            sn_sb = s0p.tile([64, SSH], f32)
            nc.sync.dma_start(sn_sb[:], sinTs_sh.ap())
            rsw = s0p.tile([64, SSH], f32)
            nc.sync.dma_start(rsw[0:32], raw_kv[32:64, 4, :])
            nc.sync.dma_start(rsw[32:64], raw_kv[0:32, 4, :])
            t1 = s0p.tile([64, SSH], f32)
            nc.vector.tensor_tensor(t1[:], raw_kv[0:64, 4, :], cs_sb[:],
                                    OP.mult)
            nc.vector.tensor_tensor(rsw[:], rsw[:], sn_sb[:], OP.mult)
            nc.vector.tensor_tensor(lat_kv[0:64, 4, :], t1[:], rsw[:], OP.add)
            nc.any.memset(lat_kv[64:128, 4, :], 0.0)
            nc.sync.dma_start(bounce_kv.ap(), lat_kv[:])

            nc.gpsimd.collective_compute(
                "AllGather", OP.bypass, replica_groups=RG,
                ins=[bounce_kv.ap().opt()], outs=[gath_kv.ap().opt()])

            # q c-tiles (sumsq deferred one tile, as above)
            raw_q = s0p.tile([128, 12, SSH], f32)
            ss_hq = s0ssp.tile([1, SSH], f32)
            pend_sq = None
            nsq = 0
            for ct in range(12):
                ps = s0ps.tile([128, SSH], f32, tag="s0ps")
                for hb in range(16):
                    mm(ps, a_q_sb[:, hb, ct * 128:(ct + 1) * 128],
                       hid_sb[:, hb, :], hb == 0, hb == 15)
                if pend_sq is not None:
                    mm(ss_hq, ones_c_fr, pend_sq, nsq == 0, nsq == 11)
                    nsq += 1
                nc.vector.tensor_copy(raw_q[:, ct, :], ps[:])
                sq = s0sqp.tile([128, SSH], f32r, tag="s0sq")
                nc.scalar.activation(sq[:], ps[:], AF.Square)
                pend_sq = sq
            mm(ss_hq, ones_c_fr, pend_sq, nsq == 0, nsq == 11)

            sq_hq = s0p.tile([1, SSH], f32)
            nc.scalar.activation(sq_hq[:], ss_hq[:], AF.Sqrt,
                                 bias=eps_sb[:], scale=1.0 / Q_LORA)
            rc_hq = s0p.tile([1, SSH], f32r)
            with nc.allow_low_precision(reason="f32r rms scale is fine"):
                nc.vector.reciprocal(rc_hq[:], sq_hq[:])
            psb_hq = s0pb.tile([128, SSH], f32, tag="s0pb")
            mm(psb_hq, ones_r_fr, rc_hq, True, True)
            bc_hq = s0p.tile([128, SSH], f32)
            nc.scalar.copy(bc_hq[:], psb_hq[:])

            lat_q = s0p.tile([128, 12, SSH], bf)
            for ct in range(12):
                nc.vector.tensor_tensor(lat_q[:, ct, :], raw_q[:, ct, :],
                                        bc_hq[:], OP.mult)
            # ACT ring: keeps this store's completion semaphore on a
            # different lane than bounce_kv's, so the kv AllGather's
            # input-ready wait doesn't serialize on this store
            nc.scalar.dma_start(bounce_q.ap(), lat_q[:])

            nc.gpsimd.collective_compute(
                "AllGather", OP.bypass, replica_groups=RG,
                ins=[bounce_q.ap().opt()], outs=[gath_q.ap().opt()])

        # ---------------- stage 1: per-head projections + attention
        with tc.tile_pool(name="s1w", bufs=1) as s1w, \
             tc.tile_pool(name="att", bufs=1) as attp:
            wqb_sb = s1w.tile([128, 12, 384], bf)
            nc.sync.dma_start(wqb_sb[:], wqbT.ap().rearrange("o p d -> p o d"))
            kpe_sb = attp.tile([64, S], bf)
            for r in range(NCORES):
                nc.sync.dma_start(kpe_sb[:, r * SSH:(r + 1) * SSH],
                                  gath_kv.ap()[r, 0:64, 4, :])

            kn_sb = s1w.tile([128, 2, S], bf)
            vt_sb = s1w.tile([128, 16, 256], bf)
            qn0 = attp.tile([128, S], bf)
            qn1 = attp.tile([128, S], bf)
            qt1 = attp.tile([128, S], f32)
            qrb = attp.tile([128, S], bf)
            qr1 = attp.tile([64, S], bf)
            cos2_sb = attp.tile([128, S], f32)
            nc.scalar.dma_start(cos2_sb[:], cosT2.ap())
            sin2_sb = attp.tile([128, S], f32)
            nc.scalar.dma_start(sin2_sb[:], sinT2s.ap())

            # k/v projections in a scoped pool so their inputs free before
            # the q path needs the SBUF
            with tc.tile_pool(name="s1kv", bufs=1) as kvp, \
                 tc.tile_pool(name="p1ps", bufs=3, space="PSUM") as p1ps, \
                 tc.tile_pool(name="p1psv", bufs=2, space="PSUM") as p1psv:
                wkvb_sb = kvp.tile([128, 4, 512], bf)
                nc.sync.dma_start(wkvb_sb[:],
                                  wkvbT.ap().rearrange("o p d -> p o d"))
                # gather load in [c_part, r, cc, s] layout (contiguous lines)
                kv_sb = kvp.tile([128, 8, 4, SSH], bf)
                for r in range(NCORES):
                    nc.sync.dma_start(kv_sb[:, r], gath_kv.ap()[r, :, 0:4, :])
                # k_nope for the 2 own heads
                for kh in range(2):
                    for tch in range(4):
                        ps = p1ps.tile([128, 512], f32, tag="p1ps")
                        for cc in range(4):
                            mm(ps, wkvb_sb[:, cc, kh * 128:(kh + 1) * 128],
                               kv_sb[:, 2 * tch:2 * tch + 2, cc, :],
                               cc == 0, cc == 3)
                        nc.scalar.copy(kn_sb[:, kh, tch * 512:(tch + 1) * 512],
                                       ps[:])
                # v^T tiles [t, 2*128]
                for tb in range(16):
                    ps = p1psv.tile([128, 256], f32, tag="p1psv")
                    for cc in range(4):
                        mm(ps, kv_sb[:, tb // 2, cc,
                                     (tb % 2) * 128:(tb % 2) * 128 + 128],
                           wkvb_sb[:, cc, 256:512], cc == 0, cc == 3)
                    nc.vector.tensor_copy(vt_sb[:, tb, :], ps[:])

            # q projections (needs the q AllGather); rope fused per chunk
            with tc.tile_pool(name="p1q", bufs=3, space="PSUM") as p1ps:
                with tc.tile_pool(name="hq", bufs=1) as hqp, \
                     tc.tile_pool(name="ropet", bufs=2) as ropep:
                    hq_sb = hqp.tile([128, 8, 12, SSH], bf)
                    for r in range(NCORES):
                        eng = nc.sync if r % 2 == 0 else nc.scalar
                        eng.dma_start(hq_sb[:, r], gath_q.ap()[r])
                    qdst = (qn0, qt1, qn1)
                    for m in range(3):
                        for tch in range(4):
                            c0, c1 = tch * 512, (tch + 1) * 512
                            ps = p1ps.tile([128, 512], f32, tag="p1ps")
                            for cc in range(12):
                                mm(ps, wqb_sb[:, cc, m * 128:(m + 1) * 128],
                                   hq_sb[:, 2 * tch:2 * tch + 2, cc, :],
                                   cc == 0, cc == 11)
                            if m == 1:
                                nc.vector.tensor_copy(qt1[:, c0:c1], ps[:])
                                # rope on this 512-token chunk
                                tmp = ropep.tile([128, 512], f32, tag="tmp")
                                for b in (0, 64):
                                    nc.sync.dma_start(
                                        tmp[b:b + 32, :],
                                        qt1[b + 32:b + 64, c0:c1])
                                    nc.sync.dma_start(
                                        tmp[b + 32:b + 64, :],
                                        qt1[b:b + 32, c0:c1])
                                nc.vector.tensor_tensor(
                                    qt1[:, c0:c1], qt1[:, c0:c1],
                                    cos2_sb[:, c0:c1], OP.mult)
                                nc.vector.tensor_tensor(
                                    tmp[:, :], tmp[:, :],
                                    sin2_sb[:, c0:c1], OP.mult)
                                nc.vector.tensor_tensor(
                                    qrb[:, c0:c1], qt1[:, c0:c1],
                                    tmp[:, :], OP.add)
                                nc.sync.dma_start(qr1[:, c0:c1],
                                                  qrb[64:128, c0:c1])
                            else:
                                nc.scalar.copy(qdst[m][:, c0:c1], ps[:])

            # full wo, loaded during attention (bulk ring)
            wo_sb = s1w.tile([128, 16, S], bf)
            nc.scalar.dma_start(wo_sb[:], woT.ap().rearrange("o p s -> p o s"))

            # attention, 2 heads, sb-paired 512-token column chunks
            with tc.tile_pool(name="apss", bufs=4, space="PSUM") as apss, \
                 tc.tile_pool(name="apsx", bufs=2, space="PSUM") as apsx, \
                 tc.tile_pool(name="apsd", bufs=2, space="PSUM") as apsd, \
                 tc.tile_pool(name="aex", bufs=6) as aexp, \
                 tc.tile_pool(name="asm", bufs=4) as asmp, \
                 tc.tile_pool(name="amk", bufs=4) as amkp, \
                 tc.tile_pool(name="xh", bufs=1) as xhp:
                for h in range(2):
                    qn_h = qn0 if h == 0 else qn1
                    qr_h = qrb if h == 0 else qr1
                    xh = xhp.tile([128, S], bf, name=f"xh{h}")
                    for sbp in range(2):
                        sA = 2 * sbp
                        sB = sA + 1
                        psx = [apsx.tile([128, 512], f32, tag="apsx",
                                         name=f"psx{j}")
                               for j in range(2)]
                        psd = [apsd.tile([1, 512], f32, tag="apsd",
                                         name=f"psd{j}")
                               for j in range(2)]
                        # software pipeline: scores for tb stream while the
                        # av/denominator matmuls consume ex[tb-1], so the PE
                        # never waits for the ACT exp
                        pend_ex = None
                        for tb in range(16):
                            pss = [apss.tile([128, 512], f32, tag="apss",
                                             name=f"pss{j}")
                                   for j in range(2)]
                            for j, sb in enumerate((sA, sB)):
                                mm(pss[j], kn_sb[:, h, tb * 128:(tb + 1) * 128],
                                   qn_h[:, sb * 512:(sb + 1) * 512],
                                   True, False)
                            for j, sb in enumerate((sA, sB)):
                                mm(pss[j], kpe_sb[:, tb * 128:(tb + 1) * 128],
                                   qr_h[0:64, sb * 512:(sb + 1) * 512],
                                   False, True)
                            if pend_ex is not None:
                                for j in range(2):
                                    mm(psx[j],
                                       vt_sb[:, tb - 1, h * 128:(h + 1) * 128],
                                       pend_ex[j], tb == 1, False)
                                for j in range(2):
                                    mm(psd[j], ones_c_bf, pend_ex[j],
                                       tb == 1, False)
                            ex = []
                            for j, sb in enumerate((sA, sB)):
                                if has_mask:
                                    mk = amkp.tile([128, 512], f32, tag="amk")
                                    nc.sync.dma_start(
                                        mk[:],
                                        maskT.ap()[tb * 128:(tb + 1) * 128,
                                                   sb * 512:(sb + 1) * 512])
                                    nc.vector.tensor_tensor(
                                        pss[j][:], pss[j][:], mk[:], OP.add)
                                e = aexp.tile([128, 512], bf, tag="aex",
                                              name=f"ex{j}")
                                nc.scalar.activation(e[:], pss[j][:], AF.Exp)
                                ex.append(e)
                            pend_ex = ex
                        for j in range(2):
                            mm(psx[j], vt_sb[:, 15, h * 128:(h + 1) * 128],
                               pend_ex[j], False, True)
                        for j in range(2):
                            mm(psd[j], ones_c_bf, pend_ex[j], False, True)
                        # batch each op type so the in-order DVE/ACT queues
                        # don't serialize the two chunks' chains
                        rds = []
                        for j in range(2):
                            rd = asmp.tile([1, 512], f32r, tag="rd",
                                           name=f"rd{j}")
                            with nc.allow_low_precision(
                                    reason="softmax denom reciprocal"):
                                nc.vector.reciprocal(rd[:], psd[j][:])
                            rds.append(rd)
                        psbs = []
                        for j in range(2):
                            psb = apss.tile([128, 512], f32, tag="apss",
                                            name=f"psbc{j}")
                            mm(psb, ones_r_fr, rds[j], True, True)
                            psbs.append(psb)
                        bcds = []
                        for j in range(2):
                            bcd = asmp.tile([128, 512], f32, tag="bcd",
                                            name=f"bcd{j}")
                            nc.scalar.copy(bcd[:], psbs[j][:])
                            bcds.append(bcd)
                        for j, sb in enumerate((sA, sB)):
                            nc.vector.tensor_tensor(
                                xh[:, sb * 512:(sb + 1) * 512], psx[j][:],
                                bcds[j][:], OP.mult)
                        bx = bounce_x0 if h == 0 else bounce_x1
                        for sb in (sA, sB):
                            for j in (2 * sb, 2 * sb + 1):
                                nc.sync.dma_start(bx.ap()[j],
                                                  xh[:, j * SSH:(j + 1) * SSH])
                    # per-head AllToAll: head 0's redistribution overlaps
                    # head 1's attention
                    nc.gpsimd.collective_compute(
                        "AllToAll", OP.bypass, replica_groups=RG,
                        ins=[bx.ap().opt()],
                        outs=[(gath_x0 if h == 0 else gath_x1).ap().opt()])

            # ---------------- output projection (token-sharded rows),
            # two passes over held PSUM groups: head-0 channels can start
            # while head-1's AllToAll is still in flight
            with tc.tile_pool(name="wops", bufs=1, space="PSUM") as wops, \
                 tc.tile_pool(name="woot", bufs=3) as wootp:
                xg0_sb = attp.tile([128, NCORES, SSH], bf)
                nc.sync.dma_start(
                    xg0_sb[:], gath_x0.ap().rearrange("r p s -> p r s"))
                pso = [wops.tile([128, 512], f32, name=f"pso{k}")
                       for k in range(8)]
                for st2 in range(2):
                    for hc in range(4):
                        for r in range(NCORES):
                            mm(pso[st2 * 4 + hc],
                               xg0_sb[:, r, st2 * 128:(st2 + 1) * 128],
                               wo_sb[:, 2 * r, hc * 512:(hc + 1) * 512],
                               r == 0, False)
                xg1_sb = attp.tile([128, NCORES, SSH], bf)
                nc.sync.dma_start(
                    xg1_sb[:], gath_x1.ap().rearrange("r p s -> p r s"))
                for st2 in range(2):
                    for hc in range(4):
                        for r in range(NCORES):
                            mm(pso[st2 * 4 + hc],
                               xg1_sb[:, r, st2 * 128:(st2 + 1) * 128],
                               wo_sb[:, 2 * r + 1, hc * 512:(hc + 1) * 512],
                               False, r == NCORES - 1)
                        ot = wootp.tile([128, 512], f32, tag="ot")
                        nc.scalar.copy(ot[:], pso[st2 * 4 + hc][:])
                        nc.sync.dma_start(
                            out.ap()[st2 * 128:(st2 + 1) * 128,
                                     hc * 512:(hc + 1) * 512], ot[:])

    nc.compile()
    return nc


def _prep_inputs(hidden_states, cos, sin, attn_mask, wq_a, q_norm_w, wq_b,
                 wkv_a, kv_norm_w, wkv_b, wo, has_mask):
    import ml_dtypes
    bf16 = ml_dtypes.bfloat16
    c = np.ascontiguousarray

    hid = np.asarray(hidden_states, np.float32)[0]          # [S, H]
    hidT = hid.T.astype(bf16)                               # [H, S]
    A_T = np.vstack([np.asarray(wq_a, np.float32),
                     np.asarray(wkv_a, np.float32)]).T      # [H, 2112]
    a_t = c(A_T.astype(bf16).reshape(16, 128, 2112))

    cosT = np.asarray(cos, np.float32).T                    # [64, S]
    sinT = np.asarray(sin, np.float32).T
    sinTs = sinT.copy()
    sinTs[0:32] *= -1.0
    cosT2 = c(np.concatenate([cosT, cosT], 0))              # [128, S]
    sinT2s = c(np.concatenate([sinTs, sinTs], 0))

    wqb = np.asarray(wq_b, np.float32) * np.asarray(q_norm_w, np.float32)[None]
    wqb = wqb * SOFTMAX_SCALE
    wkvb = (np.asarray(wkv_b, np.float32)
            * np.asarray(kv_norm_w, np.float32)[None])
    woT_full = c(np.asarray(wo, np.float32).T.astype(bf16)
                 .reshape(16, 128, S))                      # [NH*DV, H]

    qperm = np.r_[0:128, 128:192, 320:384, 192:320]
    kvperm = np.r_[0:128, 256:384, 128:256, 384:512]

    in_maps = []
    for r in range(NCORES):
        m = {
            "hidT": c(hidT[:, r * SSH:(r + 1) * SSH].reshape(16, 128, SSH)),
            "a_t": a_t,
            "cosT_sh": c(cosT[:, r * SSH:(r + 1) * SSH]),
            "sinTs_sh": c(sinTs[:, r * SSH:(r + 1) * SSH]),
            "cosT2": cosT2,
            "sinT2s": sinT2s,
            "wqbT": c(wqb[r * 384:(r + 1) * 384].T[:, qperm]
                      .astype(bf16).reshape(12, 128, 384)),
            "wkvbT": c(wkvb[r * 512:(r + 1) * 512].T[:, kvperm]
                       .astype(bf16).reshape(4, 128, 512)),
            "woT": woT_full,
            "ones_bf": np.ones((128, 1), bf16),
            "ones_fr": np.ones((128, 1), np.float32),
            "onesr_fr": np.ones((1, 128), np.float32),
        }
        if has_mask:
            m["maskT"] = c(np.asarray(attn_mask, np.float32).T)
        in_maps.append(m)
    return in_maps


def assemble(res):
    full = np.concatenate([np.asarray(res[r]["out"], np.float32)
                           for r in range(NCORES)], axis=0)
    return full.reshape(B, S, H)


def kernel(**inputs):
    from concourse.bass_utils import run_bass_kernel_spmd

    has_mask = bool(np.any(np.asarray(inputs["attn_mask"])))
    if has_mask not in _CACHE:
        _CACHE[has_mask] = _build(has_mask)
    nc = _CACHE[has_mask]

    in_maps = _prep_inputs(has_mask=has_mask, **inputs)
    res = run_bass_kernel_spmd(nc, in_maps, list(range(NCORES))).results
    return assemble(res)
